# revision 9
# baseline (speedup 1.0000x reference)
"""Causal single-head attention  B=4, T=4096, C=1024, D=64  on 8 TRN2 cores.

Sharding: 2 cores per batch; core parity p takes query rows {2i+p}.
Even/odd interleave balances causal work exactly across the pair.

Per core (all matmuls bf16, accumulate fp32):
  inputs (host-prepared):
    xt    (1024, 4096) bf16 : x[b].T with columns reordered [p::2 | 1-p::2]
    wq    (1024,   64) bf16
    wkv   (1024,  128) bf16 : [Wk | Wv]
    masks (8, 128, 512) bf16: 0/1 diagonal-block masks (4 same-parity, 4 other)
  compute:
    qT[d,i]  = Wq.T @ xt[:, :2048]            (local queries)
    kT/vT    = (Wk|Wv).T @ xt                 (packed, one pass)
    v_aug    = [v | 1] per 128-kv chunk       (PE transpose of vT)
    scoresT  = kT_chunk.T @ qT  (kv on partitions, q free -> no prob transpose)
    probs    = exp(0.125 * scoresT)           (ACT, unstabilized: |s|<~10)
    masked   = probs * mask                   (diagonal chunks only)
    accT     = sum_chunks v_aug.T @ probs     -> [65, q]  row 64 = denominator
  host: out[b, p::2] = (accT[:64] / accT[64]).T
"""

import sys

sys.path.insert(0, "/opt/trn_rl_repo")

import numpy as np
import ml_dtypes

B, T, C, D = 4, 4096, 1024, 64
TQ = T // 2            # local queries per core
NT = 4                 # q tiles of 512
QF = 512               # q free-dim tile
KC = 128               # kv chunk (partition dim of scoresT)
NCHUNK = T // KC       # 32 kv chunks per core
N_CORES = 8

_compiled = None


def _build_nc():
    import concourse.bass as bass
    import concourse.bacc as bacc
    import concourse.mybir as mybir
    from concourse.tile import TileContext
    from concourse.masks import make_identity

    f32 = mybir.dt.float32
    bf16 = mybir.dt.bfloat16

    nc = bacc.Bacc("TRN2", target_bir_lowering=False, debug=False)
    xt = nc.dram_tensor("xt", (C, T), bf16, kind="ExternalInput")
    # host pre-packs into SBUF layout: one DMA per const tile
    wq = nc.dram_tensor("wq", (128, (C // 128) * D), bf16, kind="ExternalInput")
    wkv = nc.dram_tensor("wkv", (128, (C // 128) * 2 * D), bf16, kind="ExternalInput")
    masks = nc.dram_tensor("masks", (KC, 8 * QF), bf16, kind="ExternalInput")
    out = nc.dram_tensor("out", (D + 1, TQ), f32, kind="ExternalOutput")

    NCC = C // 128  # 8 contraction chunks

    with TileContext(nc) as tc:
        with (
            tc.tile_pool(name="const", bufs=1) as constp,
            tc.tile_pool(name="xtp", bufs=66) as xtp,
            tc.tile_pool(name="ktp", bufs=9) as ktp,
            tc.tile_pool(name="vtp", bufs=3) as vtp,
            tc.tile_pool(name="qtp", bufs=5) as qtp,
            tc.tile_pool(name="vaug", bufs=33) as vaugp,
            tc.tile_pool(name="probs", bufs=6) as probsp,
            tc.tile_pool(name="osb", bufs=2) as osbp,
            tc.tile_pool(name="ps_kv", bufs=2, space="PSUM") as ps_kvp,
            tc.tile_pool(name="ps_q", bufs=1, space="PSUM") as ps_qp,
            tc.tile_pool(name="ps_s", bufs=2, space="PSUM") as ps_sp,
            tc.tile_pool(name="ps_o", bufs=1, space="PSUM") as ps_op,
            tc.tile_pool(name="ps_vt", bufs=2, space="PSUM") as ps_vtp,
        ):
            ident = constp.tile([64, 64], bf16, tag="ident")
            make_identity(nc, ident)

            wq_sb = constp.tile([128, NCC * D], bf16, tag="wq")
            wkv_sb = constp.tile([128, NCC * 2 * D], bf16, tag="wkv")
            mask_sb = constp.tile([128, 8 * QF], bf16, tag="masks")
            nc.sync.dma_start(out=wq_sb, in_=wq[:, :])
            nc.sync.dma_start(out=wkv_sb, in_=wkv[:, :])
            nc.sync.dma_start(out=mask_sb, in_=masks[:, :])

            kt_blocks = {}   # block b -> [64, 512] bf16 tile
            qt_blocks = {}   # q tile t -> [64, 512] bf16 tile
            vaug = {}        # kv chunk id -> [128, 65] bf16 tile

            # ---- projections, block order interleaves the two parity sides
            for b in (0, 4, 1, 5, 2, 6, 3, 7):
                b0 = b * QF
                xts = []
                for c in range(NCC):
                    xtile = xtp.tile([128, QF], bf16, tag="xt")
                    nc.sync.dma_start(
                        out=xtile, in_=xt[c * 128:(c + 1) * 128, b0:b0 + QF]
                    )
                    xts.append(xtile)

                ps_kv = ps_kvp.tile([128, QF], f32, tag="pskv")
                for c in range(NCC):
                    nc.tensor.matmul(
                        ps_kv,
                        lhsT=wkv_sb[:, c * 2 * D:(c + 1) * 2 * D],
                        rhs=xts[c],
                        start=(c == 0),
                        stop=(c == NCC - 1),
                    )
                kt_b = ktp.tile([64, QF], bf16, tag="kt")
                nc.vector.tensor_copy(kt_b, ps_kv[0:64, :])
                kt_blocks[b] = kt_b
                vt_b = vtp.tile([64, QF], bf16, tag="vt")
                nc.vector.tensor_copy(vt_b, ps_kv[64:128, :])

                for j in range(4):
                    ps_v = ps_vtp.tile([128, 64], bf16, tag="psvt")
                    nc.tensor.transpose(
                        ps_v, vt_b[:, j * 128:(j + 1) * 128], ident
                    )
                    va = vaugp.tile([128, D + 1], bf16, tag="vaug")
                    nc.vector.tensor_copy(va[:, 0:D], ps_v)
                    nc.gpsimd.memset(va[:, D:D + 1], 1.0)
                    vaug[b * 4 + j] = va

                if b < 4:
                    ps_q = ps_qp.tile([64, QF], f32, tag="psq")
                    for c in range(NCC):
                        nc.tensor.matmul(
                            ps_q,
                            lhsT=wq_sb[:, c * D:(c + 1) * D],
                            rhs=xts[c],
                            start=(c == 0),
                            stop=(c == NCC - 1),
                        )
                    qt_b = qtp.tile([64, QF], bf16, tag="qt")
                    nc.vector.tensor_copy(qt_b, ps_q)
                    qt_blocks[b] = qt_b

            # ---- attention
            for t in range(NT):
                qt = qt_blocks[t]
                seq = [(s, m) for m in range(4 * t + 4) for s in (0, 1)]
                ps_o = ps_op.tile([D + 1, QF], f32, tag="pso")
                for idx, (s, m) in enumerate(seq):
                    blk = s * 4 + m // 4          # xt 512-col block
                    ps_s = ps_sp.tile([128, QF], f32, tag="pss")
                    nc.tensor.matmul(
                        ps_s,
                        lhsT=kt_blocks[blk][:, (m % 4) * 128:(m % 4 + 1) * 128],
                        rhs=qt,
                        start=True,
                        stop=True,
                    )
                    p = probsp.tile([128, QF], bf16, tag="p")
                    nc.scalar.activation(
                        p, ps_s, mybir.ActivationFunctionType.Exp, scale=0.125
                    )
                    if m >= 4 * t:
                        mi = (m - 4 * t) + (0 if s == 0 else 4)
                        nc.vector.tensor_mul(
                            p, p, mask_sb[:, mi * QF:(mi + 1) * QF]
                        )
                    nc.tensor.matmul(
                        ps_o,
                        lhsT=vaug[s * 16 + m],
                        rhs=p,
                        start=(idx == 0),
                        stop=(idx == len(seq) - 1),
                        skip_group_check=True,
                    )
                o_sb = osbp.tile([D + 1, QF], f32, tag="osb")
                nc.vector.tensor_copy(o_sb, ps_o)
                nc.sync.dma_start(
                    out=out[:, t * QF:(t + 1) * QF], in_=o_sb
                )

    nc.compile()
    return nc


def _get_compiled():
    global _compiled
    if _compiled is None:
        _compiled = _build_nc()
    return _compiled


def _host_inputs(x, Wq, Wk, Wv):
    bf = ml_dtypes.bfloat16
    # pack (1024, d) weights chunk-wise along columns: (128, 8*d)
    wq = np.concatenate(
        [Wq[c * 128:(c + 1) * 128] for c in range(C // 128)], axis=1
    ).astype(bf)
    wkv_full = np.concatenate([Wk, Wv], axis=1)
    wkv = np.concatenate(
        [wkv_full[c * 128:(c + 1) * 128] for c in range(C // 128)], axis=1
    ).astype(bf)

    j = np.arange(KC)[:, None]   # kv row within chunk
    i = np.arange(QF)[None, :]   # q col within tile
    in_maps = []
    for core in range(N_CORES):
        b, p = core // 2, core % 2
        xs = x[b, p::2]          # (2048, 1024) same parity
        xo = x[b, 1 - p::2]
        xkvT = np.concatenate([xs, xo], axis=0).T
        xkvT = np.ascontiguousarray(xkvT, dtype=bf)
        ms = [(j <= i - 128 * r).astype(bf) for r in range(4)]
        mo = [(j <= i - 128 * r - (1 - p)).astype(bf) for r in range(4)]
        mask = np.concatenate(ms + mo, axis=1)   # (128, 8*512)
        in_maps.append({"xt": xkvT, "wq": wq, "wkv": wkv, "masks": mask})
    return in_maps


def kernel(x, Wq, Wk, Wv):
    from concourse.bass_utils import run_bass_kernel_spmd

    nc = _get_compiled()
    in_maps = _host_inputs(x, Wq, Wk, Wv)
    res = run_bass_kernel_spmd(nc, in_maps, core_ids=list(range(N_CORES)))

    out_full = np.empty((B, T, D), dtype=np.float32)
    for core in range(N_CORES):
        b, p = core // 2, core % 2
        acc = res.results[core]["out"]          # (65, 2048) f32
        out_full[b, p::2, :] = (acc[:D] / acc[D:D + 1]).T
    return out_full


# revision 12
# speedup vs baseline: 51.8844x; 51.8844x over previous
"""Causal single-head attention  B=4, T=4096, C=1024, D=64  on 8 TRN2 cores.

Sharding: 2 cores per batch; core parity p takes query rows {2i+p}.
Even/odd interleave balances causal work exactly across the pair.

Per core (all matmuls bf16, accumulate fp32):
  inputs (host-prepared):
    xt    (1024, 4096) bf16 : x[b].T with columns reordered [p::2 | 1-p::2]
    wq    (1024,   64) bf16
    wkv   (1024,  128) bf16 : [Wk | Wv]
    masks (8, 128, 512) bf16: 0/1 diagonal-block masks (4 same-parity, 4 other)
  compute:
    qT[d,i]  = Wq.T @ xt[:, :2048]            (local queries)
    kT/vT    = (Wk|Wv).T @ xt                 (packed, one pass)
    v_aug    = [v | 1] per 128-kv chunk       (PE transpose of vT)
    scoresT  = kT_chunk.T @ qT  (kv on partitions, q free -> no prob transpose)
    probs    = exp(0.125 * scoresT)           (ACT, unstabilized: |s|<~10)
    masked   = probs * mask                   (diagonal chunks only)
    accT     = sum_chunks v_aug.T @ probs     -> [65, q]  row 64 = denominator
  host: out[b, p::2] = (accT[:64] / accT[64]).T
"""

import sys

sys.path.insert(0, "/opt/trn_rl_repo")

import numpy as np
import ml_dtypes

B, T, C, D = 4, 4096, 1024, 64
TQ = T // 2            # local queries per core
NT = 4                 # q tiles of 512
QF = 512               # q free-dim tile
KC = 128               # kv chunk (partition dim of scoresT)
NCHUNK = T // KC       # 32 kv chunks per core
N_CORES = 8

_compiled = None


def _build_nc(loop_n=None):
    import contextlib
    import concourse.bass as bass
    import concourse.bacc as bacc
    import concourse.mybir as mybir
    from concourse.tile import TileContext
    from concourse.masks import make_identity

    f32 = mybir.dt.float32
    bf16 = mybir.dt.bfloat16

    nc = bacc.Bacc("TRN2", target_bir_lowering=False, debug=False)
    xt = nc.dram_tensor("xt", (C, T), bf16, kind="ExternalInput")
    # host pre-packs into SBUF layout: one DMA per const tile
    wq = nc.dram_tensor("wq", (128, (C // 128) * D), bf16, kind="ExternalInput")
    wkv = nc.dram_tensor("wkv", (128, (C // 128) * 2 * D), bf16, kind="ExternalInput")
    masks = nc.dram_tensor("masks", (KC, 8 * QF), bf16, kind="ExternalInput")
    out = nc.dram_tensor("out", (D + 1, TQ), f32, kind="ExternalOutput")

    NCC = C // 128  # 8 contraction chunks

    with TileContext(nc) as tc:
        with (
            tc.tile_pool(name="const", bufs=1) as constp,
            tc.tile_pool(name="xtp", bufs=66) as xtp,
            tc.tile_pool(name="ktp", bufs=9) as ktp,
            tc.tile_pool(name="vtp", bufs=3) as vtp,
            tc.tile_pool(name="qtp", bufs=5) as qtp,
            tc.tile_pool(name="vaug", bufs=33) as vaugp,
            tc.tile_pool(name="probs", bufs=6) as probsp,
            tc.tile_pool(name="osb", bufs=2) as osbp,
            tc.tile_pool(name="ps_kv", bufs=2, space="PSUM") as ps_kvp,
            tc.tile_pool(name="ps_q", bufs=1, space="PSUM") as ps_qp,
            tc.tile_pool(name="ps_s", bufs=2, space="PSUM") as ps_sp,
            tc.tile_pool(name="ps_o", bufs=1, space="PSUM") as ps_op,
            tc.tile_pool(name="ps_vt", bufs=2, space="PSUM") as ps_vtp,
        ):
            ident = constp.tile([64, 64], bf16, tag="ident")
            make_identity(nc, ident)

            wq_sb = constp.tile([128, NCC * D], bf16, tag="wq")
            wkv_sb = constp.tile([128, NCC * 2 * D], bf16, tag="wkv")
            mask_sb = constp.tile([128, 8 * QF], bf16, tag="masks")
            nc.sync.dma_start(out=wq_sb, in_=wq[:, :])
            nc.sync.dma_start(out=wkv_sb, in_=wkv[:, :])
            nc.sync.dma_start(out=mask_sb, in_=masks[:, :])

            loop_cm = (
                tc.For_i(0, loop_n, 1) if loop_n else contextlib.nullcontext()
            )
            with loop_cm:
              kt_blocks = {}   # block b -> [64, 512] bf16 tile
              qt_blocks = {}   # q tile t -> [64, 512] bf16 tile
              vaug = {}        # kv chunk id -> [128, 65] bf16 tile

              # ---- projections, block order interleaves the two parity sides
              for b in (0, 4, 1, 5, 2, 6, 3, 7):
                b0 = b * QF
                xts = []
                for c in range(NCC):
                    xtile = xtp.tile([128, QF], bf16, tag="xt")
                    nc.sync.dma_start(
                        out=xtile, in_=xt[c * 128:(c + 1) * 128, b0:b0 + QF]
                    )
                    xts.append(xtile)

                ps_kv = ps_kvp.tile([128, QF], f32, tag="pskv")
                for c in range(NCC):
                    nc.tensor.matmul(
                        ps_kv,
                        lhsT=wkv_sb[:, c * 2 * D:(c + 1) * 2 * D],
                        rhs=xts[c],
                        start=(c == 0),
                        stop=(c == NCC - 1),
                    )
                kt_b = ktp.tile([64, QF], bf16, tag="kt")
                nc.vector.tensor_copy(kt_b, ps_kv[0:64, :])
                kt_blocks[b] = kt_b
                vt_b = vtp.tile([64, QF], bf16, tag="vt")
                nc.vector.tensor_copy(vt_b, ps_kv[64:128, :])

                for j in range(4):
                    ps_v = ps_vtp.tile([128, 64], bf16, tag="psvt")
                    nc.tensor.transpose(
                        ps_v, vt_b[:, j * 128:(j + 1) * 128], ident
                    )
                    va = vaugp.tile([128, D + 1], bf16, tag="vaug")
                    nc.vector.tensor_copy(va[:, 0:D], ps_v)
                    nc.gpsimd.memset(va[:, D:D + 1], 1.0)
                    vaug[b * 4 + j] = va

                if b < 4:
                    ps_q = ps_qp.tile([64, QF], f32, tag="psq")
                    for c in range(NCC):
                        nc.tensor.matmul(
                            ps_q,
                            lhsT=wq_sb[:, c * D:(c + 1) * D],
                            rhs=xts[c],
                            start=(c == 0),
                            stop=(c == NCC - 1),
                        )
                    qt_b = qtp.tile([64, QF], bf16, tag="qt")
                    nc.vector.tensor_copy(qt_b, ps_q)
                    qt_blocks[b] = qt_b

              # ---- attention
              for t in range(NT):
                qt = qt_blocks[t]
                seq = [(s, m) for m in range(4 * t + 4) for s in (0, 1)]
                ps_o = ps_op.tile([D + 1, QF], f32, tag="pso")
                for idx, (s, m) in enumerate(seq):
                    blk = s * 4 + m // 4          # xt 512-col block
                    ps_s = ps_sp.tile([128, QF], f32, tag="pss")
                    nc.tensor.matmul(
                        ps_s,
                        lhsT=kt_blocks[blk][:, (m % 4) * 128:(m % 4 + 1) * 128],
                        rhs=qt,
                        start=True,
                        stop=True,
                    )
                    p = probsp.tile([128, QF], bf16, tag="p")
                    nc.scalar.activation(
                        p, ps_s, mybir.ActivationFunctionType.Exp, scale=0.125
                    )
                    if m >= 4 * t:
                        mi = (m - 4 * t) + (0 if s == 0 else 4)
                        nc.vector.tensor_mul(
                            p, p, mask_sb[:, mi * QF:(mi + 1) * QF]
                        )
                    nc.tensor.matmul(
                        ps_o,
                        lhsT=vaug[s * 16 + m],
                        rhs=p,
                        start=(idx == 0),
                        stop=(idx == len(seq) - 1),
                        skip_group_check=True,
                    )
                o_sb = osbp.tile([D + 1, QF], f32, tag="osb")
                nc.vector.tensor_copy(o_sb, ps_o)
                nc.sync.dma_start(
                    out=out[:, t * QF:(t + 1) * QF], in_=o_sb
                )

    nc.compile()
    return nc


def _get_compiled():
    global _compiled
    if _compiled is None:
        _compiled = _build_nc()
    return _compiled


def _host_inputs(x, Wq, Wk, Wv):
    bf = ml_dtypes.bfloat16
    # pack (1024, d) weights chunk-wise along columns: (128, 8*d)
    wq = np.concatenate(
        [Wq[c * 128:(c + 1) * 128] for c in range(C // 128)], axis=1
    ).astype(bf)
    wkv_full = np.concatenate([Wk, Wv], axis=1)
    wkv = np.concatenate(
        [wkv_full[c * 128:(c + 1) * 128] for c in range(C // 128)], axis=1
    ).astype(bf)

    j = np.arange(KC)[:, None]   # kv row within chunk
    i = np.arange(QF)[None, :]   # q col within tile
    in_maps = []
    for core in range(N_CORES):
        b, p = core // 2, core % 2
        xs = x[b, p::2]          # (2048, 1024) same parity
        xo = x[b, 1 - p::2]
        xkvT = np.concatenate([xs, xo], axis=0).T
        xkvT = np.ascontiguousarray(xkvT, dtype=bf)
        ms = [(j <= i - 128 * r).astype(bf) for r in range(4)]
        mo = [(j <= i - 128 * r - (1 - p)).astype(bf) for r in range(4)]
        mask = np.concatenate(ms + mo, axis=1)   # (128, 8*512)
        in_maps.append({"xt": xkvT, "wq": wq, "wkv": wkv, "masks": mask})
    return in_maps


def kernel(x, Wq, Wk, Wv):
    from concourse.bass_utils import run_bass_kernel_spmd

    nc = _get_compiled()
    in_maps = _host_inputs(x, Wq, Wk, Wv)
    res = run_bass_kernel_spmd(nc, in_maps, core_ids=list(range(N_CORES)))

    out_full = np.empty((B, T, D), dtype=np.float32)
    for core in range(N_CORES):
        b, p = core // 2, core % 2
        acc = res.results[core]["out"]          # (65, 2048) f32
        out_full[b, p::2, :] = (acc[:D] / acc[D:D + 1]).T
    return out_full


# revision 14
# speedup vs baseline: 77.3865x; 1.4915x over previous
"""Causal single-head attention  B=4, T=4096, C=1024, D=64  on 8 TRN2 cores.

Sharding: 2 cores per batch; core parity p takes query rows {2i+p}.
Even/odd interleave balances causal work exactly across the pair.

Per core (all matmuls bf16, accumulate fp32):
  inputs (host-prepared):
    xt    (1024, 4096) bf16 : x[b].T with columns reordered [p::2 | 1-p::2]
    wq    (1024,   64) bf16
    wkv   (1024,  128) bf16 : [Wk | Wv]
    masks (8, 128, 512) bf16: 0/1 diagonal-block masks (4 same-parity, 4 other)
  compute:
    qT[d,i]  = Wq.T @ xt[:, :2048]            (local queries)
    kT/vT    = (Wk|Wv).T @ xt                 (packed, one pass)
    v_aug    = [v | 1] per 128-kv chunk       (PE transpose of vT)
    scoresT  = kT_chunk.T @ qT  (kv on partitions, q free -> no prob transpose)
    probs    = exp(0.125 * scoresT)           (ACT, unstabilized: |s|<~10)
    masked   = probs * mask                   (diagonal chunks only)
    accT     = sum_chunks v_aug.T @ probs     -> [65, q]  row 64 = denominator
  host: out[b, p::2] = (accT[:64] / accT[64]).T
"""

import sys

sys.path.insert(0, "/opt/trn_rl_repo")

import numpy as np
import ml_dtypes

B, T, C, D = 4, 4096, 1024, 64
TQ = T // 2            # local queries per core
NT = 4                 # q tiles of 512
QF = 512               # q free-dim tile
KC = 128               # kv chunk (partition dim of scoresT)
NCHUNK = T // KC       # 32 kv chunks per core
N_CORES = 8

_compiled = None


def _build_nc(loop_n=None):
    import contextlib
    import concourse.bass as bass
    import concourse.bacc as bacc
    import concourse.mybir as mybir
    from concourse.tile import TileContext
    from concourse.masks import make_identity

    f32 = mybir.dt.float32
    bf16 = mybir.dt.bfloat16

    nc = bacc.Bacc("TRN2", target_bir_lowering=False, debug=False)
    xt = nc.dram_tensor("xt", (C, T), bf16, kind="ExternalInput")
    # host pre-packs into SBUF layout: one DMA per const tile
    wq = nc.dram_tensor("wq", (128, (C // 128) * D), bf16, kind="ExternalInput")
    wkv = nc.dram_tensor("wkv", (128, (C // 128) * 2 * D), bf16, kind="ExternalInput")
    masks = nc.dram_tensor("masks", (KC, 8 * QF), bf16, kind="ExternalInput")
    out = nc.dram_tensor("out", (D + 1, TQ), f32, kind="ExternalOutput")

    NCC = C // 128  # 8 contraction chunks

    with TileContext(nc) as tc:
        with (
            tc.tile_pool(name="const", bufs=1) as constp,
            tc.tile_pool(name="xtp", bufs=66) as xtp,
            tc.tile_pool(name="ktp", bufs=9) as ktp,
            tc.tile_pool(name="vtp", bufs=3) as vtp,
            tc.tile_pool(name="qtp", bufs=5) as qtp,
            tc.tile_pool(name="vaug", bufs=33) as vaugp,
            tc.tile_pool(name="probs", bufs=6) as probsp,
            tc.tile_pool(name="osb", bufs=2) as osbp,
            tc.tile_pool(name="ps_kv", bufs=2, space="PSUM") as ps_kvp,
            tc.tile_pool(name="ps_scr", bufs=2, space="PSUM") as ps_scrp,
            tc.tile_pool(name="ps_s", bufs=3, space="PSUM") as ps_sp,
            tc.tile_pool(name="ps_o", bufs=1, space="PSUM") as ps_op,
        ):
            ident = constp.tile([64, 64], bf16, tag="ident")
            make_identity(nc, ident)

            wq_sb = constp.tile([128, NCC * D], bf16, tag="wq")
            wkv_sb = constp.tile([128, NCC * 2 * D], bf16, tag="wkv")
            mask_sb = constp.tile([128, 8 * QF], bf16, tag="masks")
            nc.sync.dma_start(out=wq_sb, in_=wq[:, :])
            nc.sync.dma_start(out=wkv_sb, in_=wkv[:, :])
            nc.sync.dma_start(out=mask_sb, in_=masks[:, :])

            loop_cm = (
                tc.For_i(0, loop_n, 1) if loop_n else contextlib.nullcontext()
            )
            with loop_cm:
              kt_pairs = {}    # block-pair bp -> [128, 512] bf16 (rows 0:64
                               #   = same-parity kT, 64:128 = other-parity)
              qt_pairs = {}    # q tile t -> [128, 512] bf16 (rows dup)
              vaug = {}        # kv chunk id -> [128, 65] bf16 tile

              def proj_kv(b, xts):
                  ps_kv = ps_kvp.tile([128, QF], f32, tag="pskv")
                  for c in range(NCC):
                      nc.tensor.matmul(
                          ps_kv,
                          lhsT=wkv_sb[:, c * 2 * D:(c + 1) * 2 * D],
                          rhs=xts[c],
                          start=(c == 0),
                          stop=(c == NCC - 1),
                      )
                  vt_b = vtp.tile([64, QF], bf16, tag="vt")
                  nc.vector.tensor_copy(vt_b, ps_kv[64:128, :])
                  for j in range(4):
                      ps_v = ps_scrp.tile([128, 64], bf16, tag="scr")
                      nc.tensor.transpose(
                          ps_v, vt_b[:, j * 128:(j + 1) * 128], ident
                      )
                      va = vaugp.tile([128, D + 1], bf16, tag="vaug")
                      nc.vector.tensor_copy(va[:, 0:D], ps_v)
                      nc.gpsimd.memset(va[:, D:D + 1], 1.0)
                      vaug[b * 4 + j] = va
                  return ps_kv

              def load_block(b):
                  xts = []
                  for c in range(NCC):
                      xtile = xtp.tile([128, QF], bf16, tag="xt")
                      nc.sync.dma_start(
                          out=xtile,
                          in_=xt[c * 128:(c + 1) * 128, b * QF:(b + 1) * QF],
                      )
                      xts.append(xtile)
                  return xts

              # ---- projections, by block pair (same-parity, other-parity)
              for bp in range(4):
                  xts_e = load_block(bp)
                  xts_o = load_block(bp + 4)
                  ps_e = proj_kv(bp, xts_e)
                  ps_o_ = proj_kv(bp + 4, xts_o)
                  ktp_t = ktp.tile([128, QF], bf16, tag="kt")
                  nc.vector.tensor_copy(ktp_t[0:64, :], ps_e[0:64, :])
                  nc.vector.tensor_copy(ktp_t[64:128, :], ps_o_[0:64, :])
                  kt_pairs[bp] = ktp_t

                  ps_q = ps_scrp.tile([64, QF], f32, tag="scr")
                  for c in range(NCC):
                      nc.tensor.matmul(
                          ps_q,
                          lhsT=wq_sb[:, c * D:(c + 1) * D],
                          rhs=xts_e[c],
                          start=(c == 0),
                          stop=(c == NCC - 1),
                      )
                  qt_t = qtp.tile([128, QF], bf16, tag="qt")
                  nc.vector.tensor_copy(qt_t[0:64, :], ps_q)
                  nc.vector.tensor_copy(qt_t[64:128, :], ps_q)
                  qt_pairs[bp] = qt_t

              # ---- attention
              for t in range(NT):
                qt = qt_pairs[t]
                nm = 4 * t + 4
                ps_o = ps_op.tile([D + 1, QF], f32, tag="pso")
                for m in range(nm):
                    bp, j = m // 4, m % 4
                    ktp_t = kt_pairs[bp]
                    ps_a = ps_sp.tile([128, QF], f32, tag="pss")
                    ps_b = ps_sp.tile([128, QF], f32, tag="pss")
                    nc.tensor.matmul(
                        ps_a,
                        lhsT=ktp_t[0:64, j * 128:(j + 1) * 128],
                        rhs=qt[0:64, :],
                        start=True,
                        stop=True,
                    )
                    nc.tensor.matmul(
                        ps_b,
                        lhsT=ktp_t[64:128, j * 128:(j + 1) * 128],
                        rhs=qt[64:128, :],
                        start=True,
                        stop=True,
                    )
                    for s, ps_s in ((0, ps_a), (1, ps_b)):
                        p = probsp.tile([128, QF], bf16, tag="p")
                        nc.scalar.activation(
                            p, ps_s,
                            mybir.ActivationFunctionType.Exp, scale=0.125,
                        )
                        if m >= 4 * t:
                            mi = (m - 4 * t) + (0 if s == 0 else 4)
                            nc.vector.tensor_mul(
                                p, p, mask_sb[:, mi * QF:(mi + 1) * QF]
                            )
                        nc.tensor.matmul(
                            ps_o,
                            lhsT=vaug[s * 16 + m],
                            rhs=p,
                            start=(m == 0 and s == 0),
                            stop=(m == nm - 1 and s == 1),
                            skip_group_check=True,
                        )
                o_sb = osbp.tile([D + 1, QF], f32, tag="osb")
                nc.vector.tensor_copy(o_sb, ps_o)
                nc.sync.dma_start(
                    out=out[:, t * QF:(t + 1) * QF], in_=o_sb
                )

    nc.compile()
    return nc


def _get_compiled():
    global _compiled
    if _compiled is None:
        _compiled = _build_nc()
    return _compiled


def _host_inputs(x, Wq, Wk, Wv):
    bf = ml_dtypes.bfloat16
    # pack (1024, d) weights chunk-wise along columns: (128, 8*d)
    wq = np.concatenate(
        [Wq[c * 128:(c + 1) * 128] for c in range(C // 128)], axis=1
    ).astype(bf)
    wkv_full = np.concatenate([Wk, Wv], axis=1)
    wkv = np.concatenate(
        [wkv_full[c * 128:(c + 1) * 128] for c in range(C // 128)], axis=1
    ).astype(bf)

    j = np.arange(KC)[:, None]   # kv row within chunk
    i = np.arange(QF)[None, :]   # q col within tile
    in_maps = []
    for core in range(N_CORES):
        b, p = core // 2, core % 2
        xs = x[b, p::2]          # (2048, 1024) same parity
        xo = x[b, 1 - p::2]
        xkvT = np.concatenate([xs, xo], axis=0).T
        xkvT = np.ascontiguousarray(xkvT, dtype=bf)
        ms = [(j <= i - 128 * r).astype(bf) for r in range(4)]
        mo = [(j <= i - 128 * r - (1 - p)).astype(bf) for r in range(4)]
        mask = np.concatenate(ms + mo, axis=1)   # (128, 8*512)
        in_maps.append({"xt": xkvT, "wq": wq, "wkv": wkv, "masks": mask})
    return in_maps


def kernel(x, Wq, Wk, Wv):
    from concourse.bass_utils import run_bass_kernel_spmd

    nc = _get_compiled()
    in_maps = _host_inputs(x, Wq, Wk, Wv)
    res = run_bass_kernel_spmd(nc, in_maps, core_ids=list(range(N_CORES)))

    out_full = np.empty((B, T, D), dtype=np.float32)
    for core in range(N_CORES):
        b, p = core // 2, core % 2
        acc = res.results[core]["out"]          # (65, 2048) f32
        out_full[b, p::2, :] = (acc[:D] / acc[D:D + 1]).T
    return out_full


# revision 17
# speedup vs baseline: 148.2183x; 1.9153x over previous
"""Causal single-head attention  B=4, T=4096, C=1024, D=64  on 8 TRN2 cores.

Sharding: 2 cores per batch; core parity p takes query rows {2i+p}.
Even/odd interleave balances causal work exactly across the pair.

Per core (all matmuls bf16, accumulate fp32):
  inputs (host-prepared):
    xt    (1024, 4096) bf16 : x[b].T with columns reordered [p::2 | 1-p::2]
    wq    (1024,   64) bf16
    wkv   (1024,  128) bf16 : [Wk | Wv]
    masks (8, 128, 512) bf16: 0/1 diagonal-block masks (4 same-parity, 4 other)
  compute:
    qT[d,i]  = Wq.T @ xt[:, :2048]            (local queries)
    kT/vT    = (Wk|Wv).T @ xt                 (packed, one pass)
    v_aug    = [v | 1] per 128-kv chunk       (PE transpose of vT)
    scoresT  = kT_chunk.T @ qT  (kv on partitions, q free -> no prob transpose)
    probs    = exp(0.125 * scoresT)           (ACT, unstabilized: |s|<~10)
    masked   = probs * mask                   (diagonal chunks only)
    accT     = sum_chunks v_aug.T @ probs     -> [65, q]  row 64 = denominator
  host: out[b, p::2] = (accT[:64] / accT[64]).T
"""

import sys

sys.path.insert(0, "/opt/trn_rl_repo")

import numpy as np
import ml_dtypes

B, T, C, D = 4, 4096, 1024, 64
TQ = T // 2            # local queries per core
NT = 4                 # q tiles of 512
QF = 512               # q free-dim tile
KC = 128               # kv chunk (partition dim of scoresT)
NCHUNK = T // KC       # 32 kv chunks per core
N_CORES = 8

_compiled = None


def _build_nc(loop_n=None):
    import contextlib
    import concourse.bass as bass
    import concourse.bacc as bacc
    import concourse.mybir as mybir
    from concourse.tile import TileContext
    from concourse.masks import make_identity

    f32 = mybir.dt.float32
    bf16 = mybir.dt.bfloat16

    nc = bacc.Bacc("TRN2", target_bir_lowering=False, debug=False)
    xt = nc.dram_tensor("xt", (C, T), bf16, kind="ExternalInput")
    # host pre-packs into SBUF layout: one DMA per const tile
    wq = nc.dram_tensor("wq", (128, (C // 128) * D), bf16, kind="ExternalInput")
    wkv = nc.dram_tensor("wkv", (128, (C // 128) * 2 * D), bf16, kind="ExternalInput")
    masks = nc.dram_tensor("masks", (KC, 8 * QF), bf16, kind="ExternalInput")
    out = nc.dram_tensor("out", (D + 1, TQ), f32, kind="ExternalOutput")

    NCC = C // 128  # 8 contraction chunks

    with TileContext(nc) as tc:
        with (
            tc.tile_pool(name="const", bufs=1) as constp,
            tc.tile_pool(name="xtp", bufs=66) as xtp,
            tc.tile_pool(name="ktp", bufs=9) as ktp,
            tc.tile_pool(name="vtp", bufs=3) as vtp,
            tc.tile_pool(name="qtp", bufs=5) as qtp,
            tc.tile_pool(name="vaug", bufs=33) as vaugp,
            tc.tile_pool(name="probs", bufs=6) as probsp,
            tc.tile_pool(name="osb", bufs=2) as osbp,
            tc.tile_pool(name="ps_kv", bufs=1, space="PSUM") as ps_kvp,
            tc.tile_pool(name="ps_scr", bufs=2, space="PSUM") as ps_scrp,
            tc.tile_pool(name="ps_s", bufs=2, space="PSUM") as ps_sp,
            tc.tile_pool(name="ps_o", bufs=1, space="PSUM") as ps_op,
        ):
            ident = constp.tile([64, 64], bf16, tag="ident")
            make_identity(nc, ident)

            wq_sb = constp.tile([128, NCC * D], bf16, tag="wq")
            wkv_sb = constp.tile([128, NCC * 2 * D], bf16, tag="wkv")
            mask_sb = constp.tile([128, 8 * QF], bf16, tag="masks")
            nc.sync.dma_start(out=wq_sb, in_=wq[:, :])
            nc.sync.dma_start(out=wkv_sb, in_=wkv[:, :])
            nc.sync.dma_start(out=mask_sb, in_=masks[:, :])

            loop_cm = (
                tc.For_i(0, loop_n, 1) if loop_n else contextlib.nullcontext()
            )
            with loop_cm:
              kt_pairs = {}    # block-pair bp -> [128, 512] bf16 (rows 0:64
                               #   = same-parity kT, 64:128 = other-parity)
              qt_pairs = {}    # q tile t -> [128, 512] bf16 (rows dup)
              vaug = {}        # kv chunk id -> [128, 65] bf16 tile

              def proj_kv(b, xts):
                  ps_kv = ps_kvp.tile([128, QF], f32, tag="pskv")
                  for c in range(NCC):
                      nc.tensor.matmul(
                          ps_kv,
                          lhsT=wkv_sb[:, c * 2 * D:(c + 1) * 2 * D],
                          rhs=xts[c],
                          start=(c == 0),
                          stop=(c == NCC - 1),
                      )
                  vt_b = vtp.tile([64, QF], bf16, tag="vt")
                  nc.vector.tensor_copy(vt_b, ps_kv[64:128, :])
                  for j in range(4):
                      ps_v = ps_scrp.tile([128, 64], bf16, tag="scr")
                      nc.tensor.transpose(
                          ps_v, vt_b[:, j * 128:(j + 1) * 128], ident
                      )
                      va = vaugp.tile([128, D + 1], bf16, tag="vaug")
                      nc.vector.tensor_copy(va[:, 0:D], ps_v)
                      nc.gpsimd.memset(va[:, D:D + 1], 1.0)
                      vaug[b * 4 + j] = va
                  return ps_kv

              def load_block(b):
                  xts = []
                  for c in range(NCC):
                      xtile = xtp.tile([128, QF], bf16, tag="xt")
                      nc.sync.dma_start(
                          out=xtile,
                          in_=xt[c * 128:(c + 1) * 128, b * QF:(b + 1) * QF],
                      )
                      xts.append(xtile)
                  return xts

              # ---- projections, by block pair (same-parity, other-parity)
              for bp in range(4):
                  xts_e = load_block(bp)
                  xts_o = load_block(bp + 4)
                  ps_e = proj_kv(bp, xts_e)
                  ps_o_ = proj_kv(bp + 4, xts_o)
                  ktp_t = ktp.tile([128, QF], bf16, tag="kt")
                  nc.vector.tensor_copy(ktp_t[0:64, :], ps_e[0:64, :])
                  nc.vector.tensor_copy(ktp_t[64:128, :], ps_o_[0:64, :])
                  kt_pairs[bp] = ktp_t

                  ps_q = ps_scrp.tile([64, QF], f32, tag="scr")
                  for c in range(NCC):
                      nc.tensor.matmul(
                          ps_q,
                          lhsT=wq_sb[:, c * D:(c + 1) * D],
                          rhs=xts_e[c],
                          start=(c == 0),
                          stop=(c == NCC - 1),
                      )
                  qt_t = qtp.tile([128, QF], bf16, tag="qt")
                  nc.vector.tensor_copy(qt_t[0:64, :], ps_q)
                  nc.vector.tensor_copy(qt_t[64:128, :], ps_q)
                  qt_pairs[bp] = qt_t

              # ---- attention
              for t in range(NT):
                qt = qt_pairs[t]
                nm = 4 * t + 4
                ps_o = ps_op.tile([D + 1, QF], f32, tag="pso")
                for m in range(nm):
                    bp, j = m // 4, m % 4
                    ktp_t = kt_pairs[bp]
                    # q columns [0:c0] are fully masked for diagonal chunks
                    c0 = 128 * (m - 4 * t) if m >= 4 * t else 0
                    w = QF - c0
                    ps_s = ps_sp.tile([128, 2 * QF], f32, tag="pss")
                    nc.tensor.matmul(
                        ps_s[:, c0:QF],
                        lhsT=ktp_t[0:64, j * 128:(j + 1) * 128],
                        rhs=qt[0:64, c0:],
                        start=True,
                        stop=True,
                    )
                    nc.tensor.matmul(
                        ps_s[:, QF + c0:2 * QF],
                        lhsT=ktp_t[64:128, j * 128:(j + 1) * 128],
                        rhs=qt[64:128, c0:],
                        start=True,
                        stop=True,
                    )
                    p = probsp.tile([128, 2 * QF], bf16, tag="p")
                    if c0 == 0:
                        nc.scalar.activation(
                            p, ps_s,
                            mybir.ActivationFunctionType.Exp, scale=0.125,
                        )
                    else:
                        for h in (0, 1):
                            nc.scalar.activation(
                                p[:, h * QF + c0:(h + 1) * QF],
                                ps_s[:, h * QF + c0:(h + 1) * QF],
                                mybir.ActivationFunctionType.Exp, scale=0.125,
                            )
                    for s in (0, 1):
                        po = s * QF
                        if m >= 4 * t:
                            mi = (m - 4 * t) + (0 if s == 0 else 4)
                            nc.vector.tensor_mul(
                                p[:, po + c0:po + QF],
                                p[:, po + c0:po + QF],
                                mask_sb[:, mi * QF + c0:(mi + 1) * QF],
                            )
                        nc.tensor.matmul(
                            ps_o[:, c0:],
                            lhsT=vaug[s * 16 + m],
                            rhs=p[:, po + c0:po + QF],
                            start=(m == 0 and s == 0),
                            stop=(m == nm - 1 and s == 1),
                            skip_group_check=True,
                        )
                o_sb = osbp.tile([D + 1, QF], f32, tag="osb")
                nc.vector.tensor_copy(o_sb, ps_o)
                nc.sync.dma_start(
                    out=out[:, t * QF:(t + 1) * QF], in_=o_sb
                )

    nc.compile()
    return nc


def _get_compiled():
    global _compiled
    if _compiled is None:
        _compiled = _build_nc()
    return _compiled


def _host_inputs(x, Wq, Wk, Wv):
    bf = ml_dtypes.bfloat16
    # pack (1024, d) weights chunk-wise along columns: (128, 8*d)
    wq = np.concatenate(
        [Wq[c * 128:(c + 1) * 128] for c in range(C // 128)], axis=1
    ).astype(bf)
    wkv_full = np.concatenate([Wk, Wv], axis=1)
    wkv = np.concatenate(
        [wkv_full[c * 128:(c + 1) * 128] for c in range(C // 128)], axis=1
    ).astype(bf)

    j = np.arange(KC)[:, None]   # kv row within chunk
    i = np.arange(QF)[None, :]   # q col within tile
    in_maps = []
    for core in range(N_CORES):
        b, p = core // 2, core % 2
        xs = x[b, p::2]          # (2048, 1024) same parity
        xo = x[b, 1 - p::2]
        xkvT = np.concatenate([xs, xo], axis=0).T
        xkvT = np.ascontiguousarray(xkvT, dtype=bf)
        ms = [(j <= i - 128 * r).astype(bf) for r in range(4)]
        mo = [(j <= i - 128 * r - (1 - p)).astype(bf) for r in range(4)]
        mask = np.concatenate(ms + mo, axis=1)   # (128, 8*512)
        in_maps.append({"xt": xkvT, "wq": wq, "wkv": wkv, "masks": mask})
    return in_maps


def kernel(x, Wq, Wk, Wv):
    from concourse.bass_utils import run_bass_kernel_spmd

    nc = _get_compiled()
    in_maps = _host_inputs(x, Wq, Wk, Wv)
    res = run_bass_kernel_spmd(nc, in_maps, core_ids=list(range(N_CORES)))

    out_full = np.empty((B, T, D), dtype=np.float32)
    for core in range(N_CORES):
        b, p = core // 2, core % 2
        acc = res.results[core]["out"]          # (65, 2048) f32
        out_full[b, p::2, :] = (acc[:D] / acc[D:D + 1]).T
    return out_full
